# revision 1
# baseline (speedup 1.0000x reference)
"""Multi-head attention (B=2, S=2048, D=1024, H=16) on 8 trn2 NeuronCores.

Sharding: data-parallel over batch (2) x tensor-parallel over head-groups (4).
Core c handles batch c//4, heads [4*(c%4), 4*(c%4)+4).  Each core computes
Q/K/V projections for its 4 heads, attention (no mask - the reference's
causal mask is a no-op), and a partial out-projection against its slice of
Wo.  The 4 partial outputs per batch are summed on the host (+bias), which
replaces the all-reduce.

Pipeline design (v3):
  - All matmuls in float32r (fp8 was measured 3-5e-2 end-to-end error,
    over the 2e-2 tolerance; f32r at N>=256 streams 1 row/cycle, same as
    bf16, so there is no faster legal matmul mode).
  - Initial DMAs split per contraction chunk, (wq[dc], xt0[dc]) pairs
    first, so the first projection matmul starts ~1us in.
  - Attention segments emit scores with a 2-deep deferred ctx queue: the
    ctx matmuls for kt run after scores for kt+2, giving the ACT engine
    (exp is 1038ns/kt vs PE's 853ns/kt) enough slack to never head-block
    the in-order PE queue.
  - 4 of 16 exp tiles per segment run on the DVE instead of ACT via a
    Schraudolph bit-trick exp (int32(A*s+B) bitcast to float); measured
    end-to-end error contribution ~3.5e-3.  This balances ACT (12.5us/seg)
    under PE (13.7us/seg).
  - Normalization & out-projection are deferred and injected into the NEXT
    segment's score loop so segment boundaries never stall the PE.  Norm
    emission order (ctuA recipA ctuB recipB | bcA bcB | mulA mulB) keeps
    the PE broadcast matmuls off the DVE critical path.
  - PSUM->SBUF staging: Q/K proj copies on DVE, V proj copies on GpSimd,
    out-proj staging alternates GpSimd/DVE per half.
  - The last segment runs as two 256-wide halves so the final
    norm+outproj+DMA tail is half as long.
"""

import numpy as np

import concourse.mybir as mybir
from concourse import bacc
from concourse.tile import TileContext
from concourse.bass_utils import run_bass_kernel_spmd

# problem constants (hardcoded; kernel.py must be self-contained)
B, S, D, H, HD = 2, 2048, 1024, 16, 64
GROUPS = 4                 # head-groups (tensor-parallel)
HG = H // GROUPS           # heads per core = 4
DV = HG * HD               # per-core qkv width = 256
P = 128
DC = D // P                # 8 contraction chunks
ST = S // P                # 16 s/k tiles
NQ = 512                   # moving free dim / q-chunk
QC = S // NQ               # 4 q-chunks
NCORES = 8

f32 = mybir.dt.float32
f32r = mybir.dt.float32r
bf16 = mybir.dt.bfloat16
i16 = mybir.dt.int16
EXP = mybir.ActivationFunctionType.Exp

# Schraudolph exp for exp(s/8) via bf16 bit pattern (bf16 = top half of
# f32, so bits = 128*(log2(v)+127); int16 write bitcast into the bf16
# tile).  The attention-weight path (pt, v) runs in bf16: same 1 row/cycle
# matmul speed as f32r, and it sidesteps walrus's "f32r inputs must be
# f32r-rounded by their producer" rule which rejects bit-trick writes.
SCH_A16 = 128.0 * 1.4426950408889634 / 8.0
SCH_B16 = 16250.4
# Per-kt engine plan for the two exp halves (head-A, head-B):
#   X = exact exp on ACT, D = Schraudolph on DVE.
# (GpSimd cannot read PSUM on real TRN2, so it gets no exp work.)
# ACT gets 20 of 32 halves (12.2us/seg < PE 13.7us/seg); the Schraudolph
# halves are spread 6/16 per head so the ~3% bit-trick error stays uniform.
HALF_PLAN = {
    0: "XX", 1: "XD", 2: "DX", 3: "XD",
    4: "XX", 5: "DX", 6: "XD", 7: "DX",
    8: "XX", 9: "XD", 10: "DX", 11: "XD",
    12: "XX", 13: "DX", 14: "XD", 15: "DX",
}

_CACHE = {}

# timing-diagnostics only (test.py/harness always use the default)
ABLATE = frozenset()


def _build(reps=1, mode="full"):
    nc = bacc.Bacc(None, target_bir_lowering=False, debug=False)

    xt_d = nc.dram_tensor("xt", [QC, P, DC, NQ], bf16, kind="ExternalInput")
    wqt_d = nc.dram_tensor("wqt", [P, DC, DV], bf16, kind="ExternalInput")
    wkt_d = nc.dram_tensor("wkt", [P, DC, DV], bf16, kind="ExternalInput")
    wvt_d = nc.dram_tensor("wvt", [P, DC, DV], bf16, kind="ExternalInput")
    wot_d = nc.dram_tensor("wot", [P, 2, D], f32r, kind="ExternalInput")
    if "dmacoarse" in ABLATE:
        out_d = nc.dram_tensor("out", [QC, P, DC, NQ], bf16,
                               kind="ExternalOutput")
    else:
        out_d = nc.dram_tensor("out", [S, D], bf16, kind="ExternalOutput")

    from contextlib import ExitStack
    with TileContext(nc) as tc, ExitStack() as rep_ctx:
        if reps > 1:
            rep_ctx.enter_context(tc.For_i(0, reps, 1))
        with (
            tc.tile_pool(name="persist", bufs=1) as pp,
            tc.tile_pool(name="xtp", bufs=2) as xtp,
            tc.tile_pool(name="pt", bufs=8) as ptp,
            tc.tile_pool(name="osb", bufs=3) as osb,
            tc.tile_pool(name="wkp", bufs=2) as wkp,
            tc.tile_pool(name="ps2", bufs=2, space="PSUM") as ps2,
            tc.tile_pool(name="stp", bufs=4, space="PSUM") as stp,
            tc.tile_pool(name="ctp", bufs=2, space="PSUM") as ctp,
        ):
            # QT/KT tile t holds heads 2t (partitions 0:64) and 2t+1 (64:128)
            qt_sb = pp.tile([P, 2, S], f32r)
            kt_sb = pp.tile([P, 2, S], f32r)
            # V per (k-tile, head) with a ones column appended ([V | 1])
            vp_sb = pp.tile([P, ST, HG, HD + 1], bf16)
            # normalized ctx^T paired like QT/KT: tile t = heads 2t, 2t+1
            ctn_sb = pp.tile([P, 2, S], f32r)
            wot_sb = pp.tile([P, 2, D], f32r)
            ones_sb = pp.tile([P, HD], f32r)
            wq_sb = pp.tile([P, DC, DV], bf16)
            wk_sb = pp.tile([P, DC, DV], bf16)
            wv_sb = pp.tile([P, DC, DV], bf16)

            # f32r memset is rejected by the ISA, so build the ones in f32
            # and round through a DVE copy (which legally produces f32r).
            ones1 = pp.tile([P, 1], f32)
            nc.any.memset(ones1[:], 1.0)
            nc.vector.tensor_copy(ones_sb[:], ones1.broadcast_to([P, HD]))
            nc.vector.tensor_copy(
                vp_sb[:, :, :, HD:HD + 1],
                ones1.broadcast_to([P, ST, HG, 1]))

            # -------- interleaved initial DMAs ----------------------------
            xt_tiles = [None] * QC
            xt_tiles[0] = xtp.tile([P, DC, NQ], bf16, tag="xt", name="xt_sb")
            for dc in range(DC):
                nc.sync.dma_start(wq_sb[:, dc, :], wqt_d[:, dc, :])
                nc.sync.dma_start(xt_tiles[0][:, dc, :], xt_d[0, :, dc, :])
            for dc in range(DC):
                nc.sync.dma_start(wk_sb[:, dc, :], wkt_d[:, dc, :])
            for dc in range(DC):
                nc.sync.dma_start(wv_sb[:, dc, :], wvt_d[:, dc, :])
            # xt chunk DMAs are issued one iteration ahead (in the sc loop);
            # xt1 + wot right away (xtp bufs=2 -> slot free, no WAR yet)
            xt_tiles[1] = xtp.tile([P, DC, NQ], bf16, tag="xt", name="xt_sb")
            nc.sync.dma_start(xt_tiles[1][:], xt_d[1])
            nc.sync.dma_start(wot_sb[:], wot_d[:])

            # -------- attention segment machinery -------------------------
            class Seg:
                """One attention segment: heads (2t, 2t+1) x q[qlo:qlo+qw].
                Scores/exp stream per kt with a 2-deep deferred ctx queue."""

                def __init__(self, qlo, qw, t):
                    self.qlo, self.qw, self.t = qlo, qw, t
                    self.ctA = ctp.tile([P, NQ], f32, tag="ct", name="ctA")
                    self.ctB = ctp.tile([P, NQ], f32, tag="ct", name="ctB")
                    self.pending = []

                def _ctx(self, kt, pt_sb):
                    hA, hB = 2 * self.t, 2 * self.t + 1
                    w = self.qw
                    nc.tensor.matmul(
                        self.ctA[0:HD + 1, 0:w], vp_sb[:, kt, hA, :],
                        pt_sb[:, 0, 0:w], start=kt == 0, stop=kt == ST - 1)
                    nc.tensor.matmul(
                        self.ctB[0:HD + 1, 0:w], vp_sb[:, kt, hB, :],
                        pt_sb[:, 1, 0:w], start=kt == 0, stop=kt == ST - 1)

                def emit(self, kts, inject=None, depth=2):
                    qsl = slice(self.qlo, self.qlo + self.qw)
                    t, w = self.t, self.qw
                    for j, kt in enumerate(kts):
                        ksl = slice(kt * P, (kt + 1) * P)
                        stA = stp.tile([P, NQ], f32, tag="st", name="stA")
                        stB = stp.tile([P, NQ], f32, tag="st", name="stB")
                        pt_sb = ptp.tile([P, 2, NQ], bf16, tag="pt",
                                         name="pt_sb")
                        nc.tensor.matmul(
                            stA[:, 0:w], kt_sb[0:HD, t, ksl],
                            qt_sb[0:HD, t, qsl], tile_position=(0, 0))
                        nc.tensor.matmul(
                            stB[:, 0:w], kt_sb[HD:P, t, ksl],
                            qt_sb[HD:P, t, qsl], tile_position=(HD, 0))
                        for h, st_x in ((0, stA), (1, stB)):
                            typ = HALF_PLAN[kt][h]
                            if typ == "X":
                                nc.scalar.activation(
                                    pt_sb[:, h, 0:w], st_x[:, 0:w],
                                    EXP, scale=0.125)
                            else:
                                nc.vector.tensor_scalar(
                                    pt_sb.bitcast(i16)[:, h, 0:w],
                                    st_x[:, 0:w], SCH_A16, SCH_B16,
                                    mybir.AluOpType.mult,
                                    mybir.AluOpType.add)
                        if inject and j in inject:
                            for fn in inject[j]:
                                fn()
                        self.pending.append((kt, pt_sb))
                        if len(self.pending) > depth:
                            self._ctx(*self.pending.pop(0))
                    return self

                def flush(self):
                    for kt, pt_sb in self.pending:
                        self._ctx(kt, pt_sb)
                    self.pending = []
                    return self

            def norm_stage1(seg, out):
                """DVE part: stage ctx out of PSUM, reciprocal of denom."""
                w, t = seg.qw, seg.t
                for ct, h in ((seg.ctA, 2 * t), (seg.ctB, 2 * t + 1)):
                    ctu = wkp.tile([P, NQ], f32, tag="ctu", name="ctu")
                    nc.vector.tensor_copy(
                        ctu[0:HD + 1, 0:w], ct[0:HD + 1, 0:w])
                    rt = wkp.tile([P, NQ], f32r, tag="rt", name="rt")
                    with nc.allow_low_precision(
                            reason="softmax denom recip to f32r"):
                        nc.vector.reciprocal(
                            rt[HD:HD + 1, 0:w], ctu[HD:HD + 1, 0:w])
                    out.append((ctu, rt, h))

            def norm_stage2(seg, stages):
                """PE broadcast of the reciprocal + DVE normalize."""
                qsl = slice(seg.qlo, seg.qlo + seg.qw)
                w, t = seg.qw, seg.t
                bcs = []
                for ctu, rt, h in stages:
                    bc = ps2.tile([P, NQ], f32, tag="a", name="bc")
                    nc.tensor.matmul(
                        bc[0:HD, 0:w], ones_sb[HD:HD + 1, :],
                        rt[HD:HD + 1, 0:w], tile_position=(HD, 0))
                    bcs.append(bc)
                for (ctu, rt, h), bc in zip(stages, bcs):
                    if h % 2 == 0:
                        nc.vector.tensor_mul(
                            ctn_sb[0:HD, t, qsl], ctu[0:HD, 0:w],
                            bc[0:HD, 0:w])
                    else:
                        tmp = wkp.tile([P, NQ], f32r, tag="tmp", name="tmp")
                        nc.vector.tensor_mul(
                            tmp[0:HD, 0:w], ctu[0:HD, 0:w], bc[0:HD, 0:w])
                        nc.sync.dma_start(
                            ctn_sb[HD:P, t, qsl], tmp[0:HD, 0:w])

            def norm_pair(seg):
                stages = []
                norm_stage1(seg, stages)
                norm_stage2(seg, stages)

            def outproj_sti(sti, split_dma=False):
                ssl = slice(sti * P, (sti + 1) * P)
                if "noout" in ABLATE:
                    nc.sync.dma_start(out_d[ssl, :],
                                      ctn_sb[:, 0, 0:D // 2].bitcast(bf16))
                    return
                ob = osb.tile([P, D], bf16, tag="ob", name="ob")
                for ec in (0, 1):
                    esl = slice(ec * NQ, (ec + 1) * NQ)
                    op = ps2.tile([P, NQ], f32, tag="a", name="op")
                    for dvt in (0, 1):
                        nc.tensor.matmul(
                            op[:],
                            ctn_sb[:, dvt, ssl],
                            wot_sb[:, dvt, esl],
                            start=dvt == 0, stop=dvt == 1)
                    if ec == 0:
                        nc.scalar.copy(ob[:, esl], op[:])
                    else:
                        nc.vector.tensor_copy(ob[:, esl], op[:])
                    if split_dma:
                        # tail only: store each half as soon as staged so
                        # the final DMAs overlap the last copies
                        nc.sync.dma_start(out_d[ssl, esl], ob[:, esl])
                if not split_dma:
                    nc.sync.dma_start(out_d[ssl, :], ob[:])

            # -------- phase A: streamed loads + projections ---------------
            if "dmaonly" in ABLATE:
                for sc in (2, 3):
                    xt_tiles[sc] = xtp.tile([P, DC, NQ], bf16, tag="xt",
                                            name="xt_sb")
                    nc.sync.dma_start(xt_tiles[sc][:], xt_d[sc])
                for sti in range(ST):
                    nc.sync.dma_start(
                        out_d[sti * P:(sti + 1) * P, :],
                        xt_tiles[sti % 4][:, 0:2, :])
            if "dmacoarse" in ABLATE:
                # same bytes as the real kernel, minimal DMA count (11)
                for sc in (2, 3):
                    xt_tiles[sc] = xtp.tile([P, DC, NQ], bf16, tag="xt",
                                            name="xt_sb")
                    nc.sync.dma_start(xt_tiles[sc][:], xt_d[sc])
                for g in range(4):
                    nc.sync.dma_start(out_d[g], xt_tiles[g][:, :, :])

            seg00 = None
            for sc in (() if "dmaonly" in ABLATE else range(QC)):
                if seg00 is None:
                    seg00 = Seg(0, NQ, 0)
                ssl = slice(sc * NQ, (sc + 1) * NQ)
                xt_sb = xt_tiles[sc]

                def proj_qk(t):
                    for w_sb, dst in ((wq_sb, qt_sb), (wk_sb, kt_sb)):
                        ps = ps2.tile([P, NQ], f32, tag="a", name="ps")
                        for dc in range(DC):
                            nc.tensor.matmul(
                                ps[:],
                                w_sb[:, dc, t * P:(t + 1) * P],
                                xt_sb[:, dc, :],
                                start=dc == 0, stop=dc == DC - 1)
                        nc.vector.tensor_copy(dst[:, t, ssl], ps[:])

                proj_qk(0)
                proj_qk(1)
                for si in range(4):
                    sti = sc * 4 + si
                    ps = ps2.tile([P, NQ], f32, tag="a", name="ps")
                    for dc in range(DC):
                        nc.tensor.matmul(
                            ps[:, :DV],
                            xt_sb[:, dc, si * P:(si + 1) * P],
                            wv_sb[:, dc, :],
                            start=dc == 0, stop=dc == DC - 1)
                    for h in range(HG):
                        # ACT is mostly idle in phase A (GpSimd cannot
                        # read PSUM on hardware)
                        nc.scalar.copy(
                            vp_sb[:, sti, h, 0:HD],
                            ps[:, h * HD:(h + 1) * HD])
                seg00.emit(range(sc * 4, sc * 4 + 4))
                if sc + 2 < QC:
                    # prefetch chunk sc+2 (slot of sc is done being read)
                    xt_tiles[sc + 2] = xtp.tile([P, DC, NQ], bf16,
                                                tag="xt", name="xt_sb")
                    nc.sync.dma_start(xt_tiles[sc + 2][:], xt_d[sc + 2])
            if seg00 is not None:
                seg00.flush()

            # -------- phase B: pipelined attention + norm + out-proj ------
            plan = [] if "dmaonly" in ABLATE else [
                (0, NQ, 1, None),
                (NQ, NQ, 0, 0),       # inject outproj for q-chunk 0
                (NQ, NQ, 1, None),
                (2 * NQ, NQ, 0, 1),
                (2 * NQ, NQ, 1, None),
                (3 * NQ, NQ, 0, 2),
                (3 * NQ, 256, 1, None),
                (3 * NQ + 256, 256, 1, 3),   # outproj sti 12,13 only
            ]
            prev = seg00
            for qlo, qw, t, op_qc in plan:
                seg = Seg(qlo, qw, t)
                stages = []
                inject = {
                    1: [lambda s=prev, o=stages: norm_stage1(s, o)],
                    3: [lambda s=prev, o=stages: norm_stage2(s, o)],
                }
                if op_qc is not None:
                    stis = range(op_qc * 4, op_qc * 4 + 4)
                    if op_qc == 3:
                        stis = (12, 13)
                    for jj, sti in zip((6, 8, 10, 12), stis):
                        inject[jj] = [lambda s=sti: outproj_sti(s)]
                seg.emit(range(ST), inject).flush()
                prev = seg
            # tail: last half-segment's norm + final two out tiles
            if prev is not None:
                norm_pair(prev)
                outproj_sti(14, split_dma=True)
                outproj_sti(15, split_dma=True)

    nc.compile()
    return nc


def _get_nc():
    if "nc" not in _CACHE:
        _CACHE["nc"] = _build()
    return _CACHE["nc"]


def _pack_inputs(x, Wq, Wk, Wv, Wo):
    """Host-side pre-tiling into the exact DRAM layouts the NEFF expects."""
    import ml_dtypes
    bf = ml_dtypes.bfloat16
    x = np.asarray(x, np.float32)
    in_maps = []
    for c in range(NCORES):
        b, g = divmod(c, GROUPS)
        sl = slice(g * DV, (g + 1) * DV)
        xtb = np.ascontiguousarray(x[b].T)            # [D, S]
        xt = np.ascontiguousarray(
            xtb.reshape(DC, P, QC, NQ).transpose(2, 1, 0, 3)).astype(bf)
        wqt = np.ascontiguousarray(                   # [P, DC, DV]
            np.asarray(Wq, np.float32)[sl, :].T
            .reshape(DC, P, DV).transpose(1, 0, 2)).astype(bf)
        wkt = np.ascontiguousarray(
            np.asarray(Wk, np.float32)[sl, :].T
            .reshape(DC, P, DV).transpose(1, 0, 2)).astype(bf)
        wvt = np.ascontiguousarray(
            np.asarray(Wv, np.float32)[sl, :].T
            .reshape(DC, P, DV).transpose(1, 0, 2)).astype(bf)
        wot = np.ascontiguousarray(                   # [P, 2, D]
            np.asarray(Wo, np.float32)[:, sl].T
            .reshape(2, P, D).transpose(1, 0, 2))
        in_maps.append({"xt": xt, "wqt": wqt, "wkt": wkt,
                        "wvt": wvt, "wot": wot})
    return in_maps


def kernel(x, Wq, Wk, Wv, Wo, bo, _trace=False):
    bo = np.asarray(bo, np.float32)
    in_maps = _pack_inputs(x, Wq, Wk, Wv, Wo)
    res = run_bass_kernel_spmd(
        _get_nc(), in_maps, core_ids=list(range(NCORES)), trace=_trace)
    _CACHE["last_result"] = res
    parts = [np.asarray(res.results[c]["out"]).astype(np.float32)
             for c in range(NCORES)]
    out = np.empty((B, S, D), np.float32)
    for b in range(B):
        acc = np.sum(np.stack(parts[GROUPS * b:GROUPS * (b + 1)]),
                     axis=0, dtype=np.float64)
        out[b] = (acc + bo.astype(np.float64)).astype(np.float32)
    return out



# revision 2
# speedup vs baseline: 1.8283x; 1.8283x over previous
"""Multi-head attention (B=2, S=2048, D=1024, H=16) on 8 trn2 NeuronCores.

v5: swapped-ctx redesign.  Measured facts this build is shaped around
(microbench on this backend):
  - matmul wall time ~= 0.516ns x moving-cols (+~4ns), independent of
    dtype, contraction depth, and stationary reload (ldweights is free).
  - two 64-contraction-row matmuls at disjoint row quadrants run fully
    concurrently IF they target different PSUM banks.
  - matmul start=True zeroes the WHOLE psum bank; tiles are allocated
    bank-granular, so start=True is safe per-tile, but interleaved
    accumulation groups inside one tile use DVE pre-zero + start=False.
  - ACT activation ~757ns per [128,512] tile; DVE tensor_scalar
    (Schraudolph exp) ~331ns; DVE copy ~466ns.
Design:
  - scores as baseline: quadrant-paired 64-contraction matmuls -> [k,q].
  - ctx SWAPPED: stationary = 128x128 pt block, moving = [V|1] (65
    cols) -> ct[q, h, hd|den] in PSUM; 8x65-col matmuls per kt (301ns)
    instead of 2x512-col (530ns).
  - normalization via per-partition denominator: reciprocal with
    free-dim-broadcast input + one mul -> no PE broadcast matmuls.
  - ctn transposed back to [d, q] with one PE transpose per 128-q block
    (identity built on gpsimd), staged to SBUF f32r by one DVE copy.
  - exp split 12 ACT / 20 DVE-Schraudolph per 32 half-tiles (numpy
    model: ~1.25e-2 end-to-end, budget 2e-2).
  - out-proj ec-halves allocate full-bank tiles from the scores ring
    (same tile size), staging copies split ACT/DVE.
"""

import numpy as np

import concourse.mybir as mybir
from concourse import bacc
from concourse.tile import TileContext
from concourse.masks import make_identity
from concourse.bass_utils import run_bass_kernel_spmd

B, S, D, H, HD = 2, 2048, 1024, 16, 64
GROUPS = 4
HG = H // GROUPS           # heads per core = 4
DV = HG * HD               # per-core qkv width = 256
P = 128
DC = D // P                # 8 contraction chunks
ST = S // P                # 16 k tiles
NQ = 512                   # q-chunk
QC = S // NQ               # 4 q-chunks
NCORES = 8

f32 = mybir.dt.float32
f32r = mybir.dt.float32r
bf16 = mybir.dt.bfloat16
i16 = mybir.dt.int16
EXP = mybir.ActivationFunctionType.Exp

SCH_A16 = 128.0 * 1.4426950408889634 / 8.0
SCH_B16 = 16250.4

# per-kt exp engine plan: halves (A,B); X=ACT exact, D=DVE Schraudolph.
# PSUM reads throttle both engines to ~725ns/tile -> split ~17X/15D.
HALF_PLAN = {}
for _kt in range(ST):
    HALF_PLAN[_kt] = "XD" if _kt % 2 == 0 else "DX"

_CACHE = {}
import os
ABLATE = frozenset(
    x for x in os.environ.get("V5_ABLATE", "").split(",") if x)


def _build(reps=1):
    nc = bacc.Bacc(None, target_bir_lowering=False, debug=False)

    xt_d = nc.dram_tensor("xt", [QC, P, DC, NQ], bf16, kind="ExternalInput")
    wqt_d = nc.dram_tensor("wqt", [P, DC, DV], bf16, kind="ExternalInput")
    wkt_d = nc.dram_tensor("wkt", [P, DC, DV], bf16, kind="ExternalInput")
    wvt_d = nc.dram_tensor("wvt", [P, DC, DV], bf16, kind="ExternalInput")
    wot_d = nc.dram_tensor("wot", [P, 2, D], bf16, kind="ExternalInput")
    out_d = nc.dram_tensor("out", [S, D], bf16, kind="ExternalOutput")

    from contextlib import ExitStack
    with TileContext(nc) as tc, ExitStack() as stack:
        if True:
            pp = stack.enter_context(tc.tile_pool(name="persist", bufs=1))
            ident = pp.tile([P, P], bf16)
            make_identity(nc, ident[:])

            qt_sb = pp.tile([P, 2, S], bf16)
            kt_sb = pp.tile([P, 2, S], bf16)
            vp_sb = pp.tile([P, ST, HG, HD + 1], bf16)
            ctn_sb = pp.tile([P, 2, S], bf16)
            wot_sb = pp.tile([P, 2, D], bf16)
            wq_sb = pp.tile([P, DC, DV], bf16)
            wk_sb = pp.tile([P, DC, DV], bf16)
            wv_sb = pp.tile([P, DC, DV], bf16)
            if "nonorm" in ABLATE:
                nc.any.memset(ctn_sb[:], 0.1)
            ones1 = pp.tile([P, 1], f32)
            nc.any.memset(ones1[:], 1.0)
            nc.vector.tensor_copy(
                vp_sb[:, :, :, HD:HD + 1],
                ones1.broadcast_to([P, ST, HG, 1]))

        if reps > 1:
            stack.enter_context(tc.For_i(0, reps, 1))
        if True:
            xtp = stack.enter_context(tc.tile_pool(name="xtp", bufs=2))
            ptp = stack.enter_context(tc.tile_pool(name="pt", bufs=8))
            ctt = stack.enter_context(tc.tile_pool(name="ctt", bufs=2))
            osb = stack.enter_context(tc.tile_pool(name="osb", bufs=3))
            wkp = stack.enter_context(tc.tile_pool(name="wkp", bufs=2))
            stp = stack.enter_context(
                tc.tile_pool(name="stp", bufs=5, space="PSUM"))
            ctp = stack.enter_context(
                tc.tile_pool(name="ctp", bufs=2, space="PSUM"))
            tpp = stack.enter_context(
                tc.tile_pool(name="tpp", bufs=1, space="PSUM"))
            # -------- interleaved initial DMAs ----------------------------
            xt_tiles = [None] * QC
            xt_tiles[0] = xtp.tile([P, DC, NQ], bf16, tag="xt", name="xt_sb")
            for dc in range(DC):
                nc.sync.dma_start(wq_sb[:, dc, :], wqt_d[:, dc, :])
                nc.sync.dma_start(xt_tiles[0][:, dc, :], xt_d[0, :, dc, :])
            for dc in range(DC):
                nc.sync.dma_start(wk_sb[:, dc, :], wkt_d[:, dc, :])
            for dc in range(DC):
                nc.sync.dma_start(wv_sb[:, dc, :], wvt_d[:, dc, :])
            xt_tiles[1] = xtp.tile([P, DC, NQ], bf16, tag="xt", name="xt_sb")
            nc.sync.dma_start(xt_tiles[1][:], xt_d[1])
            nc.sync.dma_start(wot_sb[:], wot_d[:])

            # -------- attention segment machinery -------------------------
            class Seg:
                """Heads (2t, 2t+1) x q[qlo:qlo+qw].  Scores stream per kt
                with a 2-deep deferred swapped-ctx queue.  ct tiles hold
                [q, qb-in-tile, head, hd|den] and are zero-initialized on
                DVE (interleaved accumulation groups cannot use start)."""

                def __init__(self, qlo, qw, t, alloc_now=False):
                    self.qlo, self.qw, self.t = qlo, qw, t
                    self.nqb = qw // P
                    self.ncts = (self.nqb + 1) // 2
                    self.cts = None
                    if alloc_now:
                        self.alloc_cts()
                    self.pending = []

                def alloc_cts(self):
                    self.cts = []
                    self.ct_virgin = []
                    for i in range(self.ncts):
                        ct = ctp.tile([P, 2, 2, HD + 1], f32,
                                      tag="ct", name="ct")
                        self.cts.append(ct)
                        self.ct_virgin.append(True)

                def _ctx(self, kt, pt_sb):
                    t = self.t
                    for qb in range(self.nqb):
                        ti, sub = divmod(qb, 2)
                        for h in (0, 1):
                            # first matmul into a fresh ct tile uses
                            # start=True: the bank-wide wipe zeroes all
                            # four interleaved accumulation regions.
                            st_f = self.ct_virgin[ti]
                            self.ct_virgin[ti] = False
                            nc.tensor.matmul(
                                self.cts[ti][:, sub, h, :],
                                pt_sb[:, h, qb * P:(qb + 1) * P],
                                vp_sb[:, kt, 2 * t + h, :],
                                start=st_f, stop=kt == ST - 1,
                                skip_group_check=True)

                def emit(self, kts, inject=None, depth=4):
                    qsl = slice(self.qlo, self.qlo + self.qw)
                    t, w = self.t, self.qw
                    for j, kt in enumerate(kts):
                        ksl = slice(kt * P, (kt + 1) * P)
                        stA = stp.tile([P, NQ], f32, tag="st", name="stA")
                        stB = stp.tile([P, NQ], f32, tag="st", name="stB")
                        pt_sb = ptp.tile([P, 2, NQ], bf16, tag="pt",
                                         name="pt_sb")
                        nc.tensor.matmul(
                            stA[:, 0:w], kt_sb[0:HD, t, ksl],
                            qt_sb[0:HD, t, qsl], tile_position=(0, 0))
                        nc.tensor.matmul(
                            stB[:, 0:w], kt_sb[HD:P, t, ksl],
                            qt_sb[HD:P, t, qsl], tile_position=(HD, 0))
                        for h, st_x in ((0, stA), (1, stB)):
                            typ = HALF_PLAN[kt][h]
                            if "noexp" in ABLATE:
                                nc.gpsimd.memset(pt_sb[:, h, 0:w], 0.5)
                                continue
                            if "alldve" in ABLATE:
                                typ = "D"
                            elif "allact" in ABLATE:
                                typ = "X"
                            if typ == "X":
                                nc.scalar.activation(
                                    pt_sb[:, h, 0:w], st_x[:, 0:w],
                                    EXP, scale=0.125)
                            else:
                                nc.vector.tensor_scalar(
                                    pt_sb.bitcast(i16)[:, h, 0:w],
                                    st_x[:, 0:w], SCH_A16, SCH_B16,
                                    mybir.AluOpType.mult,
                                    mybir.AluOpType.add)
                        if inject and j in inject:
                            for fn in inject[j]:
                                fn()
                        self.pending.append((kt, pt_sb))
                        if len(self.pending) > depth:
                            self._ctx(*self.pending.pop(0))
                    return self

                def flush(self):
                    for kt, pt_sb in self.pending:
                        self._ctx(kt, pt_sb)
                    self.pending = []
                    return self

            def norm_mul(seg):
                """DVE: reciprocal of denominators + normalize -> ctn_t."""
                ctn_t = ctt.tile([P, 4, 2, HD], bf16, tag="ctn_t",
                                 name="ctn_t")
                seg.ctn_t = ctn_t
                for i, ct in enumerate(seg.cts):
                    rcp = wkp.tile([P, 2, 2, 1], f32, tag="rcp", name="rcp")
                    with nc.allow_low_precision(
                            reason="softmax denom reciprocal"):
                        nc.vector.reciprocal(
                            rcp[:], ct[:, :, :, HD:HD + 1])
                    nc.vector.tensor_mul(
                        ctn_t[:, 2 * i:2 * i + 2, :, :],
                        ct[:, :, :, 0:HD],
                        rcp.broadcast_to([P, 2, 2, HD]))

            def norm_transpose(seg):
                tp = tpp.tile([P, 4, P], bf16, tag="tp", name="tp")
                seg.tp = tp
                for qb in range(seg.nqb):
                    nc.tensor.transpose(
                        tp[:, qb, :], seg.ctn_t[:, qb, :, :], ident[:])

            def norm_stage(seg):
                qsl = slice(seg.qlo, seg.qlo + seg.qw)
                nc.vector.tensor_copy(
                    ctn_sb[:, seg.t, qsl], seg.tp[:, 0:seg.nqb, :])

            def norm_all(seg):
                norm_mul(seg)
                norm_transpose(seg)
                norm_stage(seg)

            if "nonorm" in ABLATE:
                def norm_mul(seg):        # noqa: F811
                    ctn_t = ctt.tile([P, 4, 2, HD], bf16, tag="ctn_t",
                                     name="ctn_t")
                    seg.ctn_t = ctn_t
                    rcp = wkp.tile([P, 2, 2, 1], f32, tag="rcp",
                                   name="rcp")
                    for ct in seg.cts:
                        nc.vector.reciprocal(
                            rcp[:], ct[:, :, :, HD:HD + 1])

                def norm_transpose(seg):  # noqa: F811
                    pass

                def norm_stage(seg):      # noqa: F811
                    pass

            def outproj_sti(sti, split_dma=False):
                ssl = slice(sti * P, (sti + 1) * P)
                ob = osb.tile([P, D], bf16, tag="ob", name="ob")
                if "noout" in ABLATE:
                    nc.vector.tensor_copy(
                        ob[:], ctn_sb[:, 0, 0:D // 2].bitcast(bf16))
                    nc.sync.dma_start(out_d[ssl, :], ob[:])
                    return
                for ec in (0, 1):
                    esl = slice(ec * NQ, (ec + 1) * NQ)
                    op = stp.tile([P, NQ], f32, tag="st", name="op")
                    for dvt in (0, 1):
                        nc.tensor.matmul(
                            op[:],
                            ctn_sb[:, dvt, ssl],
                            wot_sb[:, dvt, esl],
                            start=dvt == 0, stop=dvt == 1)
                    if ec == 0:
                        nc.scalar.copy(ob[:, esl], op[:])
                    else:
                        nc.vector.tensor_copy(ob[:, esl], op[:])
                    if split_dma:
                        nc.sync.dma_start(out_d[ssl, esl], ob[:, esl])
                if not split_dma:
                    nc.sync.dma_start(out_d[ssl, :], ob[:])

            # -------- phase A: streamed loads + projections ---------------
            seg00 = Seg(0, NQ, 0, alloc_now=True)
            for sc in range(QC):
                ssl = slice(sc * NQ, (sc + 1) * NQ)
                xt_sb = xt_tiles[sc]

                def proj_qk(t):
                    for w_sb, dst in ((wq_sb, qt_sb), (wk_sb, kt_sb)):
                        ps = stp.tile([P, NQ], f32, tag="st", name="ps")
                        for dc in range(DC):
                            nc.tensor.matmul(
                                ps[:],
                                w_sb[:, dc, t * P:(t + 1) * P],
                                xt_sb[:, dc, :],
                                start=dc == 0, stop=dc == DC - 1)
                        nc.vector.tensor_copy(dst[:, t, ssl], ps[:])

                proj_qk(0)
                proj_qk(1)
                for si in range(4):
                    sti = sc * 4 + si
                    ps = stp.tile([P, NQ], f32, tag="st", name="ps")
                    for dc in range(DC):
                        nc.tensor.matmul(
                            ps[:, :DV],
                            xt_sb[:, dc, si * P:(si + 1) * P],
                            wv_sb[:, dc, :],
                            start=dc == 0, stop=dc == DC - 1)
                    for h in range(HG):
                        nc.scalar.copy(
                            vp_sb[:, sti, h, 0:HD],
                            ps[:, h * HD:(h + 1) * HD])
                seg00.emit(range(sc * 4, sc * 4 + 4))
                if sc + 2 < QC:
                    xt_tiles[sc + 2] = xtp.tile([P, DC, NQ], bf16,
                                                tag="xt", name="xt_sb")
                    nc.sync.dma_start(xt_tiles[sc + 2][:], xt_d[sc + 2])
            seg00.flush()

            # -------- phase B: pipelined attention + norm + out-proj ------
            if "nophaseb" in ABLATE:
                norm_all(seg00)
                for sti in range(ST):
                    outproj_sti(sti)
                plan = []
            else:
                plan = [
                    (0, NQ, 1, None),
                    (NQ, NQ, 0, 0),
                    (NQ, NQ, 1, None),
                    (2 * NQ, NQ, 0, 1),
                    (2 * NQ, NQ, 1, None),
                    (3 * NQ, NQ, 0, 2),
                    (3 * NQ, 256, 1, None),
                    (3 * NQ + 256, 256, 1, 3),
                ]
            prev = seg00
            for qlo, qw, t, op_qc in plan:
                seg = Seg(qlo, qw, t)
                inject = {
                    1: [lambda s=prev: norm_mul(s),
                        lambda s=seg: s.alloc_cts()],
                    2: [lambda s=prev: norm_transpose(s)],
                    3: [lambda s=prev: norm_stage(s)],
                }
                if op_qc is not None:
                    stis = range(op_qc * 4, op_qc * 4 + 4)
                    if op_qc == 3:
                        stis = (12, 13)
                    for jj, sti in zip((6, 8, 10, 12), stis):
                        inject[jj] = [lambda s=sti: outproj_sti(s)]
                seg.emit(range(ST), inject).flush()
                prev = seg
            # tail
            if "nophaseb" not in ABLATE:
                norm_all(prev)
                outproj_sti(14, split_dma=True)
                outproj_sti(15, split_dma=True)

    nc.compile()
    return nc


def _get_nc():
    if "nc" not in _CACHE:
        _CACHE["nc"] = _build()
    return _CACHE["nc"]


def _pack_inputs(x, Wq, Wk, Wv, Wo):
    import ml_dtypes
    bf = ml_dtypes.bfloat16
    x = np.asarray(x, np.float32)
    in_maps = []
    for c in range(NCORES):
        b, g = divmod(c, GROUPS)
        sl = slice(g * DV, (g + 1) * DV)
        xtb = np.ascontiguousarray(x[b].T)            # [D, S]
        xt = np.ascontiguousarray(
            xtb.reshape(DC, P, QC, NQ).transpose(2, 1, 0, 3)).astype(bf)
        wqt = np.ascontiguousarray(
            np.asarray(Wq, np.float32)[sl, :].T
            .reshape(DC, P, DV).transpose(1, 0, 2)).astype(bf)
        wkt = np.ascontiguousarray(
            np.asarray(Wk, np.float32)[sl, :].T
            .reshape(DC, P, DV).transpose(1, 0, 2)).astype(bf)
        wvt = np.ascontiguousarray(
            np.asarray(Wv, np.float32)[sl, :].T
            .reshape(DC, P, DV).transpose(1, 0, 2)).astype(bf)
        wot = np.ascontiguousarray(
            np.asarray(Wo, np.float32)[:, sl].T
            .reshape(2, P, D).transpose(1, 0, 2)).astype(bf)
        in_maps.append({"xt": xt, "wqt": wqt, "wkt": wkt,
                        "wvt": wvt, "wot": wot})
    return in_maps


def kernel(x, Wq, Wk, Wv, Wo, bo, _trace=False):
    bo = np.asarray(bo, np.float32)
    in_maps = _pack_inputs(x, Wq, Wk, Wv, Wo)
    res = run_bass_kernel_spmd(
        _get_nc(), in_maps, core_ids=list(range(NCORES)), trace=_trace)
    _CACHE["last_result"] = res
    parts = [np.asarray(res.results[c]["out"]).astype(np.float32)
             for c in range(NCORES)]
    out = np.empty((B, S, D), np.float32)
    for b in range(B):
        acc = np.sum(np.stack(parts[GROUPS * b:GROUPS * (b + 1)]),
                     axis=0, dtype=np.float64)
        out[b] = (acc + bo.astype(np.float64)).astype(np.float32)
    return out
